# revision 1
# baseline (speedup 1.0000x reference)
"""DCGRU cell Trainium2 kernel: batch-parallel SPMD over 8 NeuronCores.

Sharding: data-parallel over batch B=16 -> 2 batches/core; supports and
weights replicated. No collectives.

Device algorithm per core (batches b0,b1), all matmuls bf16 w/ fp32 PSUM:
  x = concat([inputs, states], -1)                  [N, 128] per batch
  Orientation: stationary lhsT = x[m_block, d]  (natural layout),
               moving rhs = S^T[m_block, n_cols]  (host-pretransposed),
               psum out = (S@x)^T [d, n]  (feature-transposed layout).
  Phase A1: x1_s^T = (S_s @ x)^T        (stream S^T once)     -> h1T DRAM + X1 natural (via PE transpose)
  Phase A2: x2_s^T = 2*(S_s @ x1_s)^T - x^T  (stream S^T)     -> h1T DRAM
  Phase D1: ru^T = sigmoid(W_ru^T h^T + b), rs^T = r^T * states^T, XC2 natural packed
  Phase B1: x1'_s^T = (S_s @ rs)^T (both batches packed in M)  (stream S^T) -> h2T + XC3 natural
  Phase B2: x2'_s^T = 2*(S_s @ x1')^T - rs^T                   (stream S^T) -> h2T
  Phase D2: c^T = tanh(W_c^T h'^T + b_c) (reusing gconv1 inputs-half feats),
            out^T = c + u*(s - c), PE-transpose -> natural, DMA out.
"""

import sys

sys.path.insert(0, "/opt/trn_rl_repo")

from contextlib import ExitStack

import ml_dtypes
import numpy as np

import concourse.bacc as bacc
import concourse.bass as bass
import concourse.mybir as mybir
import concourse.tile as tile
from concourse.bass_utils import run_bass_kernel_spmd

BF16 = mybir.dt.bfloat16
F32 = mybir.dt.float32
AF = mybir.ActivationFunctionType
ALU = mybir.AluOpType

N = 8192
DC = 128          # D_IN + D_H
H = 64
B2 = 2            # batches per core
NBLK = N // 128   # 64 m-blocks
CH = 512          # psum chunk (free dim)
NCH = N // CH     # 16 chunks
# groups of chunks sharing one stationary load; 6 product psum banks max
GROUPS = [(0, 3), (3, 3), (6, 3), (9, 3), (12, 3), (15, 1)]
NSUP = 2

_CACHE = {}


def _build():
    import os
    PHASES = int(os.environ.get("DCGRU_PHASES", "6"))
    nc = bacc.Bacc("TRN2", target_bir_lowering=False, debug=False)

    xc_d = nc.dram_tensor("xcat", [B2, N, DC], BF16, kind="ExternalInput")
    sup_d = nc.dram_tensor("supT", [NSUP, N, N], BF16, kind="ExternalInput")
    wru_d = nc.dram_tensor("wru", [5 * DC, 2 * H], BF16, kind="ExternalInput")
    wc_d = nc.dram_tensor("wc", [5 * DC, H], BF16, kind="ExternalInput")
    bru_d = nc.dram_tensor("bru", [2 * H, 1], F32, kind="ExternalInput")
    bc_d = nc.dram_tensor("bc", [H, 1], F32, kind="ExternalInput")
    out_d = nc.dram_tensor("out", [B2, N, H], F32, kind="ExternalOutput")

    id_bf = nc.inline_tensor(np.eye(128, dtype=ml_dtypes.bfloat16), "id_bf")
    id_f = nc.inline_tensor(np.eye(128, dtype=np.float32), "id_f")

    xc_ap = xc_d.ap()
    sup_ap = sup_d.ap()
    out_ap = out_d.ap()

    with tile.TileContext(nc) as tc, ExitStack() as ctx:
        cpool = ctx.enter_context(tc.tile_pool(name="const", bufs=1))
        dram = ctx.enter_context(tc.tile_pool(name="dram", bufs=1, space="DRAM"))
        pers = ctx.enter_context(tc.tile_pool(name="pers", bufs=1))
        st = ctx.enter_context(tc.tile_pool(name="st", bufs=6))
        stage = ctx.enter_context(tc.tile_pool(name="stage", bufs=10))
        onat = ctx.enter_context(tc.tile_pool(name="onat", bufs=4))
        pp = ctx.enter_context(tc.tile_pool(name="pp", bufs=6, space="PSUM"))
        pt = ctx.enter_context(tc.tile_pool(name="pt", bufs=2, space="PSUM"))

        # ---- constants ----
        IDB = cpool.tile([128, 128], BF16, tag="idb", name="idb")
        nc.sync.dma_start(IDB[:], id_bf.ap())
        IDF = cpool.tile([128, 128], F32, tag="idf", name="idf")
        nc.sync.dma_start(IDF[:], id_f.ap())
        WRU = cpool.tile([128, 5 * 128], BF16, tag="wru", name="wru")
        nc.sync.dma_start(
            WRU[:].rearrange("p (a o) -> p a o", a=5),
            wru_d.ap().rearrange("(a p) o -> p a o", p=128),
        )
        # WC layout: cols m*64:(m+1)*64 = inputs-half block (rows 0:64);
        # cols 320+m*64 = states-half block, duplicated at rows 0:64 and 64:128
        WC = cpool.tile([128, 10 * 64], BF16, tag="wc", name="wc")
        for m in range(5):
            nc.sync.dma_start(
                WC[0:64, m * 64:(m + 1) * 64], wc_d.ap()[m * 128:m * 128 + 64, :]
            )
            nc.sync.dma_start(
                WC[0:64, 320 + m * 64:320 + (m + 1) * 64],
                wc_d.ap()[m * 128 + 64:(m + 1) * 128, :],
            )
            nc.sync.dma_start(
                WC[64:128, 320 + m * 64:320 + (m + 1) * 64],
                wc_d.ap()[m * 128 + 64:(m + 1) * 128, :],
            )
        BRU = cpool.tile([128, 1], F32, tag="bru", name="bru")
        nc.sync.dma_start(BRU[:], bru_d.ap())
        BC = cpool.tile([64, 1], F32, tag="bc", name="bc")
        nc.sync.dma_start(BC[:], bc_d.ap())

        # ---- DRAM scratch: gconv1 product feats^T (x1_s0, x2_s0, x1_s1, x2_s1) ----
        h1 = [[dram.tile([128, N], BF16, tag=f"h1_{b}_{m}", name=f"h1_{b}_{m}") for m in range(4)]
              for b in range(B2)]
        # gconv2 states-half feats^T, batch-packed rows (b*64): [x1'_s, x2'_s]
        h2 = [[dram.tile([128, N], BF16, tag=f"h2_{s}_{k}", name=f"h2_{s}_{k}") for k in range(2)]
              for s in range(NSUP)]

        # ---- persistent SBUF; tags share slots across phases by lifetime ----
        # big2: X (phase 0-A1) -> RUT (D1-end);  big4: X1 (A1-A2) -> RST/XC2/XC3
        X = [pers.tile([128, N], BF16, tag="big2", name=f"X_{b}", bufs=2)
             for b in range(B2)]
        XT = [pers.tile([128, N], BF16, tag="xt", name=f"XT_{b}", bufs=2)
              for b in range(B2)]
        X1 = [[pers.tile([128, N], BF16, tag="big4", name=f"X1_{s}_{b}", bufs=4)
               for b in range(B2)] for s in range(NSUP)]

        # ---- phase 0: load x natural, make x^T ----
        for b in range(B2):
            nc.sync.dma_start(
                X[b][:].rearrange("p (a d) -> p a d", a=NBLK),
                xc_ap[b].rearrange("(a p) d -> p a d", p=128),
            )
            for nb in range(NBLK):
                ps = pt.tile([128, 128], BF16, tag="tp", name="tp")
                nc.tensor.transpose(
                    ps[:], X[b][:, nb * 128:(nb + 1) * 128], IDB[:]
                )
                nc.vector.tensor_copy(XT[b][:, nb * 128:(nb + 1) * 128], ps[:])

        def product_stream(lhs_of, psum_sink, pack_batches):
            """Stream supT once; for each (s, group, m_block) do matmuls.

            lhs_of(s, b, mb) -> lhsT AP [128, 128] (or packed [128,128] when
            pack_batches). psum_sink(s, b_or_None, j, c0, cnt, psum) consumes
            the finished [128, CH] f32 psum for chunk c0+j.
            """
            for s in range(NSUP):
                for (c0, cnt) in GROUPS:
                    gc = cnt * CH
                    if pack_batches:
                        psums = [pp.tile([128, CH], F32, tag="pp", name="pp") for j in range(cnt)]
                    else:
                        psums = [pp.tile([128, CH], F32, tag="pp", name="pp")
                                 for _ in range(B2 * cnt)]
                    for mb in range(NBLK):
                        stt = st.tile([128, gc], BF16, tag="st", name="st")
                        nc.sync.dma_start(
                            stt[:],
                            sup_ap[s, mb * 128:(mb + 1) * 128,
                                   c0 * CH:c0 * CH + gc],
                        )
                        first = mb == 0
                        last = mb == NBLK - 1
                        if pack_batches:
                            lhsT = lhs_of(s, None, mb)
                            for j in range(cnt):
                                nc.tensor.matmul(
                                    psums[j][:], lhsT,
                                    stt[:, j * CH:(j + 1) * CH],
                                    start=first, stop=last,
                                )
                        else:
                            for b in range(B2):
                                lhsT = lhs_of(s, b, mb)
                                for j in range(cnt):
                                    nc.tensor.matmul(
                                        psums[b * cnt + j][:], lhsT,
                                        stt[:, j * CH:(j + 1) * CH],
                                        start=first, stop=last,
                                    )
                    if pack_batches:
                        for j in range(cnt):
                            psum_sink(s, None, j, c0, cnt, psums[j])
                    else:
                        for b in range(B2):
                            for j in range(cnt):
                                psum_sink(s, b, j, c0, cnt, psums[b * cnt + j])

        # ---- A1: x1_s^T = (S_s @ x)^T ----
        def a1_sink(s, b, j, c0, cnt, psum):
            cc = c0 + j
            cols = slice(cc * CH, (cc + 1) * CH)
            t = stage.tile([128, CH], BF16, tag="sg", name="sg")
            nc.vector.tensor_copy(t[:], psum[:])
            nc.sync.dma_start(h1[b][2 * s][:, cols], t[:])
            for tp in range(4):
                blk = cc * 4 + tp
                ps = pt.tile([128, 128], BF16, tag="tp", name="tp")
                nc.tensor.transpose(ps[:], t[:, tp * 128:(tp + 1) * 128], IDB[:])
                nc.vector.tensor_copy(
                    X1[s][b][:, blk * 128:(blk + 1) * 128], ps[:]
                )

        product_stream(lambda s, b, mb: X[b][:, mb * 128:(mb + 1) * 128],
                       a1_sink, pack_batches=False)

        if PHASES < 2:
            return nc
        # ---- A2: x2_s^T = 2*(S_s @ x1_s)^T - x^T ----
        def a2_sink(s, b, j, c0, cnt, psum):
            cc = c0 + j
            cols = slice(cc * CH, (cc + 1) * CH)
            t = stage.tile([128, CH], BF16, tag="sg", name="sg")
            nc.vector.scalar_tensor_tensor(
                t[:], psum[:], 2.0, XT[b][:, cols],
                op0=ALU.mult, op1=ALU.subtract,
            )
            nc.sync.dma_start(h1[b][2 * s + 1][:, cols], t[:])

        product_stream(lambda s, b, mb: X1[s][b][:, mb * 128:(mb + 1) * 128],
                       a2_sink, pack_batches=False)

        if PHASES < 3:
            return nc
        # ---- D1: dense ru + sigmoid + rs^T + XC2 natural ----
        RUT = [pers.tile([128, N], BF16, tag="big2", name=f"RUT_{b}", bufs=2)
               for b in range(B2)]
        RST = pers.tile([128, N], BF16, tag="big4", name="RST", bufs=4)
        XC2 = pers.tile([128, N], BF16, tag="big4", name="XC2", bufs=4)
        for b in range(B2):
            for cc in range(NCH):
                cols = slice(cc * CH, (cc + 1) * CH)
                ps = pt.tile([128, CH], F32, tag="tp", name="tp")
                for i in range(5):
                    if i == 0:
                        rhs = XT[b][:, cols]
                    else:
                        sg = stage.tile([128, CH], BF16, tag="sg", name="sg")
                        nc.sync.dma_start(sg[:], h1[b][i - 1][:, cols])
                        rhs = sg[:]
                    nc.tensor.matmul(
                        ps[:], WRU[:, i * 128:(i + 1) * 128], rhs,
                        start=(i == 0), stop=(i == 4),
                    )
                nc.scalar.activation(
                    RUT[b][:, cols], ps[:], AF.Sigmoid, bias=BRU[:]
                )
                # rs = r * states^T; base-shift states^T and the result via
                # single-input copies (SB-SB two-input ops need equal bases)
                sts = stage.tile([64, CH], BF16, tag="sh1", name="sh1", bufs=3)
                nc.vector.tensor_copy(sts[:], XT[b][64:128, cols])
                rsc = stage.tile([64, CH], BF16, tag="sh2", name="sh2", bufs=3)
                nc.vector.tensor_mul(rsc[:], RUT[b][0:64, cols], sts[:])
                nc.vector.tensor_copy(RST[b * 64:(b + 1) * 64, cols], rsc[:])
                for tp in range(4):
                    blk = cc * 4 + tp
                    ps2 = pt.tile([128, 128], BF16, tag="tp", name="tp")
                    nc.tensor.transpose(
                        ps2[0:128, 0:64],
                        RST[b * 64:(b + 1) * 64, blk * 128:(blk + 1) * 128],
                        IDB[b * 64:(b + 1) * 64, b * 64:b * 64 + 64],
                    )
                    nc.vector.tensor_copy(
                        XC2[:, blk * 128 + b * 64:blk * 128 + b * 64 + 64],
                        ps2[0:128, 0:64],
                    )

        if PHASES < 4:
            return nc
        # ---- B1: x1'_s^T packed = (S_s @ rs)^T ----
        XC3 = pers.tile([128, N], BF16, tag="big4", name="XC3", bufs=4)

        def b1_sink(s, b, j, c0, cnt, psum):
            cc = c0 + j
            cols = slice(cc * CH, (cc + 1) * CH)
            t = stage.tile([128, CH], BF16, tag="sg", name="sg")
            nc.vector.tensor_copy(t[:], psum[:])
            nc.sync.dma_start(h2[s][0][:, cols], t[:])
            for tp in range(4):
                blk = cc * 4 + tp
                ps = pt.tile([128, 128], BF16, tag="tp", name="tp")
                nc.tensor.transpose(ps[:], t[:, tp * 128:(tp + 1) * 128], IDB[:])
                nc.vector.tensor_copy(
                    XC3[:, blk * 128:(blk + 1) * 128], ps[:]
                )

        product_stream(lambda s, b, mb: XC2[:, mb * 128:(mb + 1) * 128],
                       b1_sink, pack_batches=True)

        if PHASES < 5:
            return nc
        # ---- B2: x2'_s^T packed = 2*(S_s @ x1')^T - rs^T ----
        def b2_sink(s, b, j, c0, cnt, psum):
            cc = c0 + j
            cols = slice(cc * CH, (cc + 1) * CH)
            t = stage.tile([128, CH], BF16, tag="sg", name="sg")
            nc.vector.scalar_tensor_tensor(
                t[:], psum[:], 2.0, RST[:, cols],
                op0=ALU.mult, op1=ALU.subtract,
            )
            nc.sync.dma_start(h2[s][1][:, cols], t[:])

        product_stream(lambda s, b, mb: XC3[:, mb * 128:(mb + 1) * 128],
                       b2_sink, pack_batches=True)

        if PHASES < 6:
            return nc
        # ---- D2: dense c + tanh + blend + transpose + out ----
        for b in range(B2):
            for cc in range(NCH):
                cols = slice(cc * CH, (cc + 1) * CH)
                ps = pt.tile([128, CH], F32, tag="tp", name="tp")
                pc = ps[0:64, :]
                nmm = 0
                for m in range(5):
                    # inputs-half: lhsT at rows 0:64, rhs at base 0
                    if m == 0:
                        rhs_i = XT[b][0:64, cols]
                    else:
                        sg = stage.tile([128, CH], BF16, tag="sg", name="sg")
                        nc.sync.dma_start(sg[0:64, :], h1[b][m - 1][0:64, cols])
                        rhs_i = sg[0:64, :]
                    nc.tensor.matmul(
                        pc, WC[0:64, m * 64:(m + 1) * 64], rhs_i,
                        start=(nmm == 0), stop=False,
                    )
                    nmm += 1
                    # states-half: stage everything at base 0 so every matmul
                    # keeps tile_position (0,0)
                    if m == 0:
                        sgr = stage.tile([64, CH], BF16, tag="sgr", name="sgr",
                                         bufs=3)
                        nc.vector.tensor_copy(
                            sgr[:], RST[b * 64:(b + 1) * 64, cols]
                        )
                        rhs_s = sgr[:]
                    else:
                        s_idx = (m - 1) // 2
                        k_idx = (m - 1) % 2
                        sg = stage.tile([128, CH], BF16, tag="sg", name="sg")
                        nc.sync.dma_start(
                            sg[0:64, :],
                            h2[s_idx][k_idx][b * 64:(b + 1) * 64, cols],
                        )
                        rhs_s = sg[0:64, :]
                    lhs_s = WC[0:64, 320 + m * 64:320 + (m + 1) * 64]
                    nmm += 1
                    nc.tensor.matmul(
                        pc, lhs_s, rhs_s, start=False, stop=(nmm == 10),
                    )
                ctf = stage.tile([64, CH], F32, tag="f1", name="f1", bufs=3)
                nc.scalar.activation(ctf[:], pc, AF.Tanh, bias=BC[:])
                sts = stage.tile([64, CH], F32, tag="f5", name="f5", bufs=3)
                nc.vector.tensor_copy(sts[:], XT[b][64:128, cols])
                uts = stage.tile([64, CH], F32, tag="f6", name="f6", bufs=3)
                nc.vector.tensor_copy(uts[:], RUT[b][64:128, cols])
                t1 = stage.tile([64, CH], F32, tag="f2", name="f2", bufs=3)
                nc.vector.tensor_sub(t1[:], sts[:], ctf[:])
                t2 = stage.tile([64, CH], F32, tag="f3", name="f3", bufs=3)
                nc.vector.tensor_mul(t2[:], t1[:], uts[:])
                otf = stage.tile([64, CH], F32, tag="f4", name="f4", bufs=3)
                nc.vector.tensor_add(otf[:], ctf[:], t2[:])
                for tp in range(4):
                    blk = cc * 4 + tp
                    pso = pt.tile([128, 128], F32, tag="tp", name="tp")
                    nc.tensor.transpose(
                        pso[0:128, 0:64],
                        otf[:, tp * 128:(tp + 1) * 128],
                        IDF[0:64, 0:64],
                    )
                    ont = onat.tile([128, 64], F32, tag="on", name="on")
                    nc.vector.tensor_copy(ont[:], pso[0:128, 0:64])
                    nc.sync.dma_start(
                        out_ap[b, blk * 128:(blk + 1) * 128, :], ont[:]
                    )

    return nc


def _get_nc():
    if "nc" not in _CACHE:
        nc = _build()
        nc.compile()
        _CACHE["nc"] = nc
    return _CACHE["nc"]


def kernel(inputs, states, supports, W_ru, b_ru, W_c, b_c, _trace=False):
    bf = ml_dtypes.bfloat16
    B = inputs.shape[0]
    ncore = 8
    bper = B // ncore

    x_cat = np.concatenate([inputs, states], axis=-1).astype(bf)     # [16,N,128]
    supT = np.ascontiguousarray(
        np.asarray(supports).transpose(0, 2, 1)).astype(bf)          # [2,N,N]
    wru = np.asarray(W_ru).astype(bf)
    wc = np.asarray(W_c).astype(bf)
    bru = np.asarray(b_ru).astype(np.float32).reshape(2 * H, 1)
    bc = np.asarray(b_c).astype(np.float32).reshape(H, 1)

    nc = _get_nc()
    in_maps = []
    for c in range(ncore):
        in_maps.append({
            "xcat": np.ascontiguousarray(x_cat[c * bper:(c + 1) * bper]),
            "supT": supT,
            "wru": wru,
            "wc": wc,
            "bru": bru,
            "bc": bc,
        })
    res = run_bass_kernel_spmd(
        nc, in_maps, core_ids=list(range(ncore)), trace=_trace,
    )
    outs = [r["out"] for r in res.results]
    full = np.concatenate(outs, axis=0).astype(np.float32)           # [16,N,64]
    if _trace:
        kernel.last_results = res
    return full, full



# revision 6
# speedup vs baseline: 1.8780x; 1.8780x over previous
"""DCGRU cell Trainium2 kernel: batch-parallel SPMD over 8 NeuronCores.

Sharding: data-parallel over batch B=16 -> 2 batches/core; supports and
weights replicated. No collectives.

The 4 diffusion product streams (A1/A2 for gconv1, B1/B2 for gconv2)
dominate: each streams S^T once. They run in fp8e4m3 with DoubleRow
perf mode (contraction of two 128-node blocks per matmul), halving both
HBM traffic (64MB/support/stream) and PE time vs bf16. S is scaled by
2^12 before fp8 quantization (its values ~2^-12 would flush to zero);
hop-1 outputs are re-quantized to fp8 with a 2^6 scale. Dense phases
(D1 ru/sigmoid, D2 c/tanh/blend) stay bf16.

Orientation: stationary lhsT = x[m-dblock, d] fp8 natural layout,
moving rhs = S^T[m-dblock, n_cols] fp8, psum = (S@x)^T [d, n] f32.

Per-core phases:
  0:  XT = x^T via DMA-transpose; X8 = fp8 x natural (host-cast)
  A1: x1_s^T = (S_s@x)^T        -> h1 DRAM bf16 + X1q natural fp8
  A2: x2_s^T = 2(S_s@x1)^T - x^T -> h1
  D1: ru^T = sigmoid(W_ru^T h^T + b); rs^T; XC2 = rs natural fp8 packed
  B1: x1'_s^T = (S_s@rs)^T packed -> h2 + XC3 natural fp8
  B2: x2'_s^T = 2(S_s@x1')^T - rs^T -> h2
  D2: c^T = tanh(W_c^T h'^T + b_c), out^T = c + u*(s - c), PE-transpose,
      DMA out. (inputs-half feats of gconv2 reuse gconv1's h1 rows 0:64)
"""

import sys

sys.path.insert(0, "/opt/trn_rl_repo")

from contextlib import ExitStack

import ml_dtypes
import numpy as np

import concourse.bacc as bacc
import concourse.bass as bass
import concourse.mybir as mybir
import concourse.tile as tile
from concourse.bass_utils import run_bass_kernel_spmd

BF16 = mybir.dt.bfloat16
F32 = mybir.dt.float32
F8 = mybir.dt.float8e4
AF = mybir.ActivationFunctionType
ALU = mybir.AluOpType
DR = mybir.MatmulPerfMode.DoubleRow

N = 8192
DC = 128          # D_IN + D_H
H = 64
B2 = 2            # batches per core
NBLK = N // 128   # 64 m-blocks
ND = NBLK // 2    # 32 double m-blocks (DoubleRow)
CH = 512          # psum chunk (free dim)
NCH = N // CH     # 16 chunks
# groups of chunks sharing one stationary load; 6 product psum banks max
GROUPS = [(0, 3), (3, 3), (6, 3), (9, 3), (12, 3), (15, 1)]
NSUP = 2

S_SC = 2.0 ** 12   # host scale on S before fp8 quantization
X_SC = 2.0 ** 6    # scale on hop-1 outputs for fp8 re-quantization

_CACHE = {}


def _build():
    import os
    PHASES = int(os.environ.get("DCGRU_PHASES", "6"))
    nc = bacc.Bacc("TRN2", target_bir_lowering=False, debug=False)

    xc_d = nc.dram_tensor("xcat", [B2, N, DC], BF16, kind="ExternalInput")
    xc8_d = nc.dram_tensor("xcat8", [B2, N, DC], F8, kind="ExternalInput")
    sup_d = nc.dram_tensor("supT", [NSUP, N, N], F8, kind="ExternalInput")
    wru_d = nc.dram_tensor("wru", [5 * DC, 2 * H], BF16, kind="ExternalInput")
    wc_d = nc.dram_tensor("wc", [5 * DC, H], BF16, kind="ExternalInput")
    bru_d = nc.dram_tensor("bru", [2 * H, 1], F32, kind="ExternalInput")
    bc_d = nc.dram_tensor("bc", [H, 1], F32, kind="ExternalInput")
    out_d = nc.dram_tensor("out", [B2, N, H], F32, kind="ExternalOutput")

    id_bf = nc.inline_tensor(np.eye(128, dtype=ml_dtypes.bfloat16), "id_bf")
    id_f = nc.inline_tensor(np.eye(128, dtype=np.float32), "id_f")
    id_8 = nc.inline_tensor(np.eye(128, dtype=ml_dtypes.float8_e4m3), "id_8")

    xc_ap = xc_d.ap()
    xc8_ap = xc8_d.ap()
    sup_ap = sup_d.ap()
    out_ap = out_d.ap()

    with tile.TileContext(nc) as tc, ExitStack() as ctx:
        cpool = ctx.enter_context(tc.tile_pool(name="const", bufs=1))
        dram = ctx.enter_context(tc.tile_pool(name="dram", bufs=1, space="DRAM"))
        pers = ctx.enter_context(tc.tile_pool(name="pers", bufs=1))
        st = ctx.enter_context(tc.tile_pool(name="st", bufs=6))
        stage = ctx.enter_context(tc.tile_pool(name="stage", bufs=10))
        onat = ctx.enter_context(tc.tile_pool(name="onat", bufs=4))
        pp = ctx.enter_context(tc.tile_pool(name="pp", bufs=6, space="PSUM"))
        pt = ctx.enter_context(tc.tile_pool(name="pt", bufs=2, space="PSUM"))

        # ---- constants ----
        IDB = cpool.tile([128, 128], BF16, tag="idb", name="idb")
        nc.sync.dma_start(IDB[:], id_bf.ap())
        IDF = cpool.tile([128, 128], F32, tag="idf", name="idf")
        nc.sync.dma_start(IDF[:], id_f.ap())
        ID8 = cpool.tile([128, 128], F8, tag="id8", name="id8")
        nc.sync.dma_start(ID8[:], id_8.ap())
        WRU = cpool.tile([128, 5 * 128], BF16, tag="wru", name="wru")
        nc.sync.dma_start(
            WRU[:].rearrange("p (a o) -> p a o", a=5),
            wru_d.ap().rearrange("(a p) o -> p a o", p=128),
        )
        # WC layout: cols m*64:(m+1)*64 = inputs-half block (rows 0:64);
        # cols 320+m*64 = states-half block, duplicated at rows 0:64 and 64:128
        WC = cpool.tile([128, 10 * 64], BF16, tag="wc", name="wc")
        for m in range(5):
            nc.sync.dma_start(
                WC[0:64, m * 64:(m + 1) * 64], wc_d.ap()[m * 128:m * 128 + 64, :]
            )
            nc.sync.dma_start(
                WC[0:64, 320 + m * 64:320 + (m + 1) * 64],
                wc_d.ap()[m * 128 + 64:(m + 1) * 128, :],
            )
            nc.sync.dma_start(
                WC[64:128, 320 + m * 64:320 + (m + 1) * 64],
                wc_d.ap()[m * 128 + 64:(m + 1) * 128, :],
            )
        BRU = cpool.tile([128, 1], F32, tag="bru", name="bru")
        nc.sync.dma_start(BRU[:], bru_d.ap())
        BC = cpool.tile([64, 1], F32, tag="bc", name="bc")
        nc.sync.dma_start(BC[:], bc_d.ap())

        # ---- DRAM scratch: gconv1 product feats^T (x1_s0, x2_s0, x1_s1, x2_s1) ----
        h1 = [[dram.tile([128, N], BF16, tag=f"h1_{b}_{m}", name=f"h1_{b}_{m}") for m in range(4)]
              for b in range(B2)]
        # gconv2 states-half feats^T, batch-packed rows (b*64): [x1'_s, x2'_s]
        h2 = [[dram.tile([128, N], BF16, tag=f"h2_{s}_{k}", name=f"h2_{s}_{k}") for k in range(2)]
              for s in range(NSUP)]

        # ---- persistent SBUF ----
        XT = [pers.tile([128, N], BF16, tag="xt", name=f"XT_{b}", bufs=2)
              for b in range(B2)]
        X8 = [pers.tile([128, N], F8, tag="x8", name=f"X8_{b}", bufs=2)
              for b in range(B2)]
        X1q = [[pers.tile([128, N], F8, tag="x1q", name=f"X1q_{s}_{b}", bufs=4)
                for b in range(B2)] for s in range(NSUP)]

        # ---- phase 0: x^T via DMA transpose, x natural fp8 from host ----
        for b in range(B2):
            nc.sync.dma_start_transpose(XT[b][:], xc_ap[b])
            nc.sync.dma_start(
                X8[b][:].rearrange("p (a d) -> p a d", a=NBLK),
                xc8_ap[b].rearrange("(a p) d -> p a d", p=128),
            )

        def dr_slice(T, a2):
            """[128, 2, 128] DoubleRow lhsT view of natural-layout tile T."""
            return T[:, a2 * 256:(a2 + 1) * 256].rearrange(
                "p (k d) -> p k d", k=2)

        def product_stream(lhs_of, psum_sink, pack_batches):
            """Stream supT once (fp8, DoubleRow over double m-blocks).

            lhs_of(s, b, a2) -> lhsT AP [128, 2, 128]. psum_sink(s,
            b_or_None, j, c0, cnt, psum) consumes the finished [128, CH]
            f32 psum for chunk c0+j.
            """
            for s in range(NSUP):
                for (c0, cnt) in GROUPS:
                    gc = cnt * CH
                    if pack_batches:
                        psums = [pp.tile([128, CH], F32, tag="pp", name="pp") for j in range(cnt)]
                    else:
                        psums = [pp.tile([128, CH], F32, tag="pp", name="pp")
                                 for _ in range(B2 * cnt)]
                    for a2 in range(ND):
                        stt = st.tile([128, 2 * gc], F8, tag="st", name="st")
                        nc.sync.dma_start(
                            stt[:].rearrange("p (k c) -> p k c", k=2),
                            sup_ap[s, a2 * 256:(a2 + 1) * 256,
                                   c0 * CH:c0 * CH + gc].rearrange(
                                       "(k p) c -> p k c", p=128),
                        )
                        rhs3 = stt[:].rearrange("p (k c) -> p k c", k=2)
                        first = a2 == 0
                        last = a2 == ND - 1
                        if pack_batches:
                            lhsT = lhs_of(s, None, a2)
                            for j in range(cnt):
                                nc.tensor.matmul(
                                    psums[j][:], lhsT,
                                    rhs3[:, :, j * CH:(j + 1) * CH],
                                    start=first, stop=last, perf_mode=DR,
                                )
                        else:
                            for b in range(B2):
                                lhsT = lhs_of(s, b, a2)
                                for j in range(cnt):
                                    nc.tensor.matmul(
                                        psums[b * cnt + j][:], lhsT,
                                        rhs3[:, :, j * CH:(j + 1) * CH],
                                        start=first, stop=last, perf_mode=DR,
                                    )
                    if pack_batches:
                        for j in range(cnt):
                            psum_sink(s, None, j, c0, cnt, psums[j])
                    else:
                        for b in range(B2):
                            for j in range(cnt):
                                psum_sink(s, b, j, c0, cnt, psums[b * cnt + j])

        def hop1_sink(h_dst, q_dst):
            """psum = 2^12 (S@x): h_dst gets unscaled bf16 ^T feats, q_dst
            gets fp8 2^6-scaled natural layout via PE transposes."""
            def sink(s, b, j, c0, cnt, psum):
                cc = c0 + j
                cols = slice(cc * CH, (cc + 1) * CH)
                t = stage.tile([128, CH], BF16, tag="sg", name="sg")
                nc.scalar.activation(t[:], psum[:], AF.Copy, scale=1.0 / S_SC)
                nc.sync.dma_start(h_dst(s, b)[:, cols], t[:])
                t8 = stage.tile([128, CH], F8, tag="s8", name="s8", bufs=3)
                nc.vector.tensor_scalar_mul(t8[:], psum[:], X_SC / S_SC)
                for tp in range(4):
                    blk = cc * 4 + tp
                    ps8 = pt.tile([128, 256], F8, tag="tp", name="tp")
                    ps8_s = ps8[:].rearrange("p (c two) -> p c two", two=2)[:, :, 0]
                    nc.tensor.transpose(
                        ps8_s, t8[:, tp * 128:(tp + 1) * 128], ID8[:]
                    )
                    nc.vector.tensor_copy(
                        q_dst(s, b)[:, blk * 128:(blk + 1) * 128], ps8_s
                    )
            return sink

        def hop2_sink(h_dst, sub_of):
            """psum = 2^18 (S@x1): x2 = psum*2^-17 - sub (bf16 ^T)."""
            def sink(s, b, j, c0, cnt, psum):
                cc = c0 + j
                cols = slice(cc * CH, (cc + 1) * CH)
                t = stage.tile([128, CH], BF16, tag="sg", name="sg")
                nc.vector.scalar_tensor_tensor(
                    t[:], psum[:], 2.0 / (S_SC * X_SC), sub_of(b)[:, cols],
                    op0=ALU.mult, op1=ALU.subtract,
                )
                nc.sync.dma_start(h_dst(s, b)[:, cols], t[:])
            return sink

        # ---- A1: x1_s^T = (S_s @ x)^T ----
        product_stream(
            lambda s, b, a2: dr_slice(X8[b], a2),
            hop1_sink(lambda s, b: h1[b][2 * s], lambda s, b: X1q[s][b]),
            pack_batches=False,
        )

        if PHASES < 2:
            return nc
        # ---- A2: x2_s^T = 2*(S_s @ x1_s)^T - x^T ----
        product_stream(
            lambda s, b, a2: dr_slice(X1q[s][b], a2),
            hop2_sink(lambda s, b: h1[b][2 * s + 1], lambda b: XT[b]),
            pack_batches=False,
        )

        if PHASES < 3:
            return nc
        # ---- D1: dense ru + sigmoid + rs^T + XC2 natural fp8 ----
        RUT = [pers.tile([128, N], BF16, tag="big2", name=f"RUT_{b}", bufs=2)
               for b in range(B2)]
        RST = pers.tile([128, N], BF16, tag="rst", name="RST")
        # XC2/XC3 reuse X8's two fp8 slots (X8 is dead after A1)
        XC2 = pers.tile([128, N], F8, tag="x8", name="XC2", bufs=2)
        for b in range(B2):
            for cc in range(NCH):
                cols = slice(cc * CH, (cc + 1) * CH)
                ps = pt.tile([128, CH], F32, tag="tp", name="tp")
                for i in range(5):
                    if i == 0:
                        rhs = XT[b][:, cols]
                    else:
                        sg = stage.tile([128, CH], BF16, tag="sg", name="sg")
                        nc.sync.dma_start(sg[:], h1[b][i - 1][:, cols])
                        rhs = sg[:]
                    nc.tensor.matmul(
                        ps[:], WRU[:, i * 128:(i + 1) * 128], rhs,
                        start=(i == 0), stop=(i == 4),
                    )
                nc.scalar.activation(
                    RUT[b][:, cols], ps[:], AF.Sigmoid, bias=BRU[:]
                )
                # rs = r * states^T; base-shift states^T and the result via
                # single-input copies (SB-SB two-input ops need equal bases)
                sts = stage.tile([64, CH], BF16, tag="sh1", name="sh1", bufs=3)
                nc.vector.tensor_copy(sts[:], XT[b][64:128, cols])
                rsc = stage.tile([64, CH], BF16, tag="sh2", name="sh2", bufs=3)
                nc.vector.tensor_mul(rsc[:], RUT[b][0:64, cols], sts[:])
                nc.vector.tensor_copy(RST[b * 64:(b + 1) * 64, cols], rsc[:])
                for tp in range(4):
                    blk = cc * 4 + tp
                    ps2 = pt.tile([128, 128], BF16, tag="tp", name="tp")
                    nc.tensor.transpose(
                        ps2[0:128, 0:64],
                        RST[b * 64:(b + 1) * 64, blk * 128:(blk + 1) * 128],
                        IDB[b * 64:(b + 1) * 64, b * 64:b * 64 + 64],
                    )
                    nc.vector.tensor_copy(
                        XC2[:, blk * 128 + b * 64:blk * 128 + b * 64 + 64],
                        ps2[0:128, 0:64],
                    )

        if PHASES < 4:
            return nc
        # ---- B1: x1'_s^T packed = (S_s @ rs)^T ----
        XC3 = pers.tile([128, N], F8, tag="x8", name="XC3", bufs=2)
        product_stream(
            lambda s, b, a2: dr_slice(XC2, a2),
            hop1_sink(lambda s, b: h2[s][0], lambda s, b: XC3),
            pack_batches=True,
        )

        if PHASES < 5:
            return nc
        # ---- B2: x2'_s^T packed = 2*(S_s @ x1')^T - rs^T ----
        product_stream(
            lambda s, b, a2: dr_slice(XC3, a2),
            hop2_sink(lambda s, b: h2[s][1], lambda b: RST),
            pack_batches=True,
        )

        if PHASES < 6:
            return nc
        # ---- D2: dense c + tanh + blend + transpose + out ----
        for b in range(B2):
            for cc in range(NCH):
                cols = slice(cc * CH, (cc + 1) * CH)
                ps = pt.tile([128, CH], F32, tag="tp", name="tp")
                pc = ps[0:64, :]
                nmm = 0
                for m in range(5):
                    # inputs-half: lhsT at rows 0:64, rhs at base 0
                    if m == 0:
                        rhs_i = XT[b][0:64, cols]
                    else:
                        sg = stage.tile([128, CH], BF16, tag="sg", name="sg")
                        nc.sync.dma_start(sg[0:64, :], h1[b][m - 1][0:64, cols])
                        rhs_i = sg[0:64, :]
                    nc.tensor.matmul(
                        pc, WC[0:64, m * 64:(m + 1) * 64], rhs_i,
                        start=(nmm == 0), stop=False,
                    )
                    nmm += 1
                    # states-half: stage everything at base 0 so every matmul
                    # keeps tile_position (0,0)
                    if m == 0:
                        sgr = stage.tile([64, CH], BF16, tag="sgr", name="sgr",
                                         bufs=3)
                        nc.vector.tensor_copy(
                            sgr[:], RST[b * 64:(b + 1) * 64, cols]
                        )
                        rhs_s = sgr[:]
                    else:
                        s_idx = (m - 1) // 2
                        k_idx = (m - 1) % 2
                        sg = stage.tile([128, CH], BF16, tag="sg", name="sg")
                        nc.sync.dma_start(
                            sg[0:64, :],
                            h2[s_idx][k_idx][b * 64:(b + 1) * 64, cols],
                        )
                        rhs_s = sg[0:64, :]
                    lhs_s = WC[0:64, 320 + m * 64:320 + (m + 1) * 64]
                    nmm += 1
                    nc.tensor.matmul(
                        pc, lhs_s, rhs_s, start=False, stop=(nmm == 10),
                    )
                ctf = stage.tile([64, CH], F32, tag="f1", name="f1", bufs=2)
                nc.scalar.activation(ctf[:], pc, AF.Tanh, bias=BC[:])
                sts = stage.tile([64, CH], F32, tag="f5", name="f5", bufs=2)
                nc.vector.tensor_copy(sts[:], XT[b][64:128, cols])
                uts = stage.tile([64, CH], F32, tag="f6", name="f6", bufs=2)
                nc.vector.tensor_copy(uts[:], RUT[b][64:128, cols])
                t1 = stage.tile([64, CH], F32, tag="f2", name="f2", bufs=2)
                nc.vector.tensor_sub(t1[:], sts[:], ctf[:])
                t2 = stage.tile([64, CH], F32, tag="f3", name="f3", bufs=2)
                nc.vector.tensor_mul(t2[:], t1[:], uts[:])
                otf = stage.tile([64, CH], F32, tag="f4", name="f4", bufs=2)
                nc.vector.tensor_add(otf[:], ctf[:], t2[:])
                for tp in range(4):
                    blk = cc * 4 + tp
                    pso = pt.tile([128, 128], F32, tag="tp", name="tp")
                    nc.tensor.transpose(
                        pso[0:128, 0:64],
                        otf[:, tp * 128:(tp + 1) * 128],
                        IDF[0:64, 0:64],
                    )
                    ont = onat.tile([128, 64], F32, tag="on", name="on")
                    nc.vector.tensor_copy(ont[:], pso[0:128, 0:64])
                    nc.sync.dma_start(
                        out_ap[b, blk * 128:(blk + 1) * 128, :], ont[:]
                    )

    return nc


def _get_nc():
    if "nc" not in _CACHE:
        nc = _build()
        nc.compile()
        _CACHE["nc"] = nc
    return _CACHE["nc"]


def kernel(inputs, states, supports, W_ru, b_ru, W_c, b_c, _trace=False):
    bf = ml_dtypes.bfloat16
    f8 = ml_dtypes.float8_e4m3
    B = inputs.shape[0]
    ncore = 8
    bper = B // ncore

    x_cat32 = np.concatenate([inputs, states], axis=-1)              # [16,N,128]
    x_cat = x_cat32.astype(bf)
    x_cat8 = x_cat32.astype(f8)
    supT8 = (np.asarray(supports).transpose(0, 2, 1) * S_SC).astype(f8)
    supT8 = np.ascontiguousarray(supT8)                              # [2,N,N]
    wru = np.asarray(W_ru).astype(bf)
    wc = np.asarray(W_c).astype(bf)
    bru = np.asarray(b_ru).astype(np.float32).reshape(2 * H, 1)
    bc = np.asarray(b_c).astype(np.float32).reshape(H, 1)

    nc = _get_nc()
    in_maps = []
    for c in range(ncore):
        in_maps.append({
            "xcat": np.ascontiguousarray(x_cat[c * bper:(c + 1) * bper]),
            "xcat8": np.ascontiguousarray(x_cat8[c * bper:(c + 1) * bper]),
            "supT": supT8,
            "wru": wru,
            "wc": wc,
            "bru": bru,
            "bc": bc,
        })
    res = run_bass_kernel_spmd(
        nc, in_maps, core_ids=list(range(ncore)), trace=_trace,
    )
    outs = [r["out"] for r in res.results]
    full = np.concatenate(outs, axis=0).astype(np.float32)           # [16,N,64]
    if _trace:
        kernel.last_results = res
    return full, full
